# revision 1
# baseline (speedup 1.0000x reference)
"""Trainium2 Bass kernel for nn_AttBlock (BS=4, C=64, H=W=64).

Math (per sample b, x viewed as [64 ch, 4096 px]):
  v  = v_w @ x + v_b                  [64, 4096]
  k  = k_w @ x + k_b                  [8, 4096]
  ql = gl*(q_l_w @ x + q_l_b)         [8, 4096]   (gamma_l folded in)
  A[c,e] = sum_q v[c,q] * gg*qg_w[q,e]            [64, 8]
  s[c]   = sum_q v[c,q] * gg*qg_b[q]              [64]
  B[c,e] = sum_q v[c,q] * k[e,q]                  [64, 8]
  out[c,p] = sum_e A[c,e]*k_g[p,e] + s[c] + sum_e B[c,e]*ql[e,p] + x[c,p]
  where k_g[p,e] = k.flat[8p+e]  (the torch .view reinterpretation)

The [4096,4096] alphas matrices are never materialized: associativity
collapses the attention into rank-8/9 factor matmuls, so the kernel is
memory-bound (read x, write out).

Sharding: core i = (b = i//2, h = i%2): sample b, output pixel half h.
All per-core differences (sample, half, the 4 k-channel rows the k_g
view needs) live in per-core *data*, so one SPMD graph serves all 8
cores. No collectives: each core reads its full sample (1 MiB).

k_g.T build (the mod-8 lane shuffle): kch [4,4096] --SBUF DMA-->
T1[r, 32j+m] = kch[j, 32r+m] --PE transpose--> T2[32j+m, r] (PSUM)
--16 strided copies--> r_g[e, 512j+4r+w] = T2[32j+8w+e, r].
PSUM sources may start at any partition (SBUF APs must start 32-aligned,
PSUM APs are unconstrained), which is what makes the copies legal.
"""

import numpy as np

BS, C, HW = 4, 64, 4096
C8 = 8
HALF = HW // 2  # 2048

_CACHE = {}

# consts column layout
QGT0 = 0      # [128, 288]: qg tiled; tile t at cols 9t:9t+9 (col 8 of each = gg*qg_b)
VW0 = 288     # [65, 72]:  [v_w.T; v_b] ++ [k_w.T; k_b]   (fused vk conv weight)
K40 = 360     # [65, 4]:   per-core [k_w.T[:, 4h:4h+4]; k_b[4h:4h+4]]
QL0 = 364     # [65, 8]:   gl*[q_l_w.T; q_l_b]
ID0 = 448     # [128, 128]: identity


def _build_graph():
    import concourse.bass as bass
    import concourse.mybir as mybir
    from concourse.tile import TileContext

    FP32 = mybir.dt.float32
    nc = bass.Bass("TRN2", target_bir_lowering=False)

    # ---- DRAM parameters (per-core shards) ----
    xg_d = nc.declare_dram_parameter("xg", [C + 1, HW], FP32, isOutput=False)
    xh_d = nc.declare_dram_parameter("xh", [C + 1, HALF], FP32, isOutput=False)
    cn_d = nc.declare_dram_parameter("consts", [128, 576], FP32, isOutput=False)
    out_d = nc.declare_dram_parameter("out", [C, HALF], FP32, isOutput=True)

    with TileContext(nc) as tc:
        with (
            tc.tile_pool(name="big", bufs=1) as big,
            tc.tile_pool(name="vkp", bufs=3) as vkp,
            tc.tile_pool(name="psg", bufs=1, space="PSUM") as psg,
            tc.tile_pool(name="psl", bufs=1, space="PSUM") as psl,
            tc.tile_pool(name="psvk", bufs=2, space="PSUM") as psvk,
            tc.tile_pool(name="psql", bufs=1, space="PSUM") as psql,
            tc.tile_pool(name="pskc", bufs=1, space="PSUM") as pskc,
            tc.tile_pool(name="psout", bufs=2, space="PSUM") as psout,
        ):
            x_lo = big.tile([C + 1, HALF], FP32, tag="x_lo")   # x cols 0:2048 + ones row
            x_hi = big.tile([C + 1, HALF], FP32, tag="x_hi")   # x cols 2048:4096 + ones row
            c_sb = big.tile([128, 576], FP32, tag="c_sb")
            kch = big.tile([4, HW], FP32, tag="kch")           # k rows 4h:4h+4, all px
            t1 = big.tile([128, 128], FP32, tag="t1")          # kch re-blocked
            r_ql = big.tile([C8, HALF], FP32, tag="r_ql")      # ql at own-half px
            r_g = big.tile([9, HALF], FP32, tag="r_g")         # rows 0:8 k_g.T, row 8 ones
            r_x = big.tile([C + 1, HALF], FP32, tag="r_x")     # own-half x + ones row
            pg_sb = big.tile([9, C], FP32, tag="pg_sb")        # [A.T; s]
            pl_sb = big.tile([C8, C], FP32, tag="pl_sb")       # B.T
            o_sb = big.tile([C, HALF], FP32, tag="o_sb")

            # ---- loads ----
            nc.sync.dma_start(out=c_sb[:, :], in_=cn_d[:, :])
            nc.sync.dma_start(out=x_lo[:, :], in_=xg_d[:, 0:HALF])
            nc.sync.dma_start(out=x_hi[:, :], in_=xg_d[:, HALF:HW])
            nc.sync.dma_start(out=r_x[:, :], in_=xh_d[:, :])
            # row 8 stays 1.0 (the 'ones' row); rows 0:8 are fully
            # overwritten by the k_g.T copies below. (A direct memset of
            # r_g[8:9] is illegal: SBUF writes must start 32-aligned.)
            nc.vector.memset(r_g[:, :], 1.0)

            xs = [x_lo, x_hi]

            # Tiny first matmul reading only c_sb: walrus allows a single
            # sync-wait on the LDWEIGHTS struct, so each PE instruction
            # may introduce at most ONE new semaphore. This observes the
            # consts DMA lane so the first real matmul waits only on x.
            dmy = psout.tile([1, 1], FP32, tag="op")
            nc.tensor.matmul(dmy[0:1, 0:1], c_sb[0:1, 0:1], c_sb[0:1, 0:1],
                             start=True, stop=True)
            nc.tensor.matmul(dmy[0:1, 0:1], x_hi[0:1, 0:1], x_hi[0:1, 0:1],
                             start=True, stop=True, skip_group_check=True)

            # ---- ch-major convs ----
            # the 4 k rows the k_g view needs, at ALL 4096 px (global order)
            for u in range(8):
                xt = xs[u // 4][:, (u % 4) * 512:(u % 4) * 512 + 512]
                kc = pskc.tile([4, 512], FP32, tag="kc")
                nc.tensor.matmul(kc[:, :], c_sb[0:65, K40:K40 + 4], xt,
                                 start=True, stop=True)
                nc.vector.tensor_copy(kch[:, u * 512:(u + 1) * 512], kc[:, :])
            # ql at own-half px (rhs = r_x includes ones row -> bias applied)
            for u in range(4):
                qc = psql.tile([C8, 512], FP32, tag="qc")
                nc.tensor.matmul(qc[:, :], c_sb[0:65, QL0:QL0 + 8],
                                 r_x[:, u * 512:(u + 1) * 512], start=True, stop=True)
                nc.scalar.copy(r_ql[:, u * 512:(u + 1) * 512], qc[:, :])

            # ---- k_g.T build ----
            # T1[r, 32j+m] = kch[j, 32r+m]   (SBUF->SBUF DMA, 128B leaves)
            # (one DMA per j so the partition dim stays outermost in every AP)
            for j in range(4):
                nc.gpsimd.dma_start(
                    out=t1[:, 32 * j:32 * j + 32],
                    in_=kch[j:j + 1, :].rearrange("o (r m) -> o r m", m=32))
            # Per (j, w): transpose T1[:, 32j+8w : +8] -> [8, 128] PSUM
            # (free-dim offsets are unconstrained; partition starts must
            # be 32-aligned, so slice T1 on the free axis only), then
            # copy to r_g[e, 512j+4r+w] with a stride-4 free-dim dst.
            for j in range(4):
                if j > 0:
                    # observe T1-DMA lane j alone (see the dummy-matmul
                    # note above: one new semaphore per PE instruction)
                    nc.tensor.matmul(dmy[0:1, 0:1], t1[0:1, 32 * j:32 * j + 1],
                                     t1[0:1, 32 * j:32 * j + 1],
                                     start=True, stop=True, skip_group_check=True)
                for wp in range(2):
                    t2 = psout.tile([C8, 256], FP32, tag="op")
                    for s2 in range(2):
                        w = 2 * wp + s2
                        co = 32 * j + 8 * w
                        nc.tensor.transpose(t2[:, 128 * s2:128 * s2 + 128],
                                            t1[:, co:co + 8], c_sb[:, ID0:ID0 + 128])
                    # dst cols 512j + 4r + 2wp + s2:
                    # [e][r: step 4, count 128][s2: step 1, count 2]
                    dstw = r_g[0:8, 512 * j:512 * j + 512].rearrange(
                        "e (r s) -> e r s", s=4)[:, :, 2 * wp:2 * wp + 2]
                    # src cols 128*s2 + r: [r: step 1][s2: step 128]
                    srcw = t2[:, :].rearrange("e (s r) -> e r s", r=128)
                    nc.vector.tensor_copy(dstw, srcw)

            # ---- px-major pipeline: vk tiles + factor accumulation ----
            pt_g = psg.tile([9, C], FP32, tag="ptg")
            pt_l = psl.tile([C8, C], FP32, tag="ptl")
            for p2 in range(16):
                vk = psvk.tile([128, 144], FP32, tag="vk")
                vks = vkp.tile([128, 144], FP32, tag="vks")
                for s2 in range(2):
                    t = 2 * p2 + s2
                    xt = xs[t // 16][:, (t % 16) * 128:(t % 16) * 128 + 128]
                    nc.tensor.matmul(vk[:, 72 * s2:72 * s2 + 72], xt,
                                     c_sb[0:65, VW0:VW0 + 72],
                                     start=True, stop=True)
                nc.vector.tensor_copy(vks[:, :], vk[:, :])
                for s2 in range(2):
                    t = 2 * p2 + s2
                    o = 72 * s2
                    nc.tensor.matmul(pt_g[:, :], c_sb[:, 9 * t:9 * t + 9],
                                     vks[:, o:o + 64], start=(t == 0), stop=(t == 31))
                    nc.tensor.matmul(pt_l[:, :], vks[:, o + 64:o + 72],
                                     vks[:, o:o + 64], start=(t == 0), stop=(t == 31))

            nc.vector.tensor_copy(pg_sb[:, :], pt_g[:, :])
            nc.vector.tensor_copy(pl_sb[:, :], pt_l[:, :])

            # ---- final: out = Pg.T @ [k_g.T; ones] + Pl.T @ ql + I @ x_own ----
            for u in range(4):
                op = psout.tile([C, 512], FP32, tag="op")
                sl = slice(u * 512, (u + 1) * 512)
                nc.tensor.matmul(op[:, :], pg_sb[:, :], r_g[:, sl],
                                 start=True, stop=False)
                nc.tensor.matmul(op[:, :], pl_sb[:, :], r_ql[:, sl],
                                 start=False, stop=False)
                nc.tensor.matmul(op[:, :], c_sb[0:C, ID0:ID0 + C], r_x[0:C, sl],
                                 start=False, stop=True)
                nc.scalar.copy(o_sb[:, sl], op[:, :])
            nc.sync.dma_start(out=out_d[:, 0:1024], in_=o_sb[:, 0:1024])
            nc.sync.dma_start(out=out_d[:, 1024:2048], in_=o_sb[:, 1024:2048])

    _strip_dead_waits(nc)
    return nc


def _strip_dead_waits(nc):
    """Remove semaphore waits already implied by same-engine program order.

    Tile's dead-wait elimination pass (optimize_sems) is disabled, but
    walrus only encodes ONE sync-wait per compute instruction, so any
    instruction carrying {cross-engine wait, same-engine dead wait} fails
    codegen. A wait on sem S with value <= (number of increments to S by
    *synchronously-completing* instructions earlier on the same engine) is
    trivially satisfied when the instruction issues (engines complete
    in-order; semaphores are monotone), so it can be dropped. DMA
    increments are excluded: DMA completions are asynchronous.
    """
    from collections import defaultdict

    sync_compute = {
        "InstMatmult", "InstTensorCopy", "InstActivation", "InstMemset",
        "InstTensorTensor", "InstTensorScalarPtr", "InstTensorReduce",
        "InstCopy", "InstStreamTranspose", "InstIota", "InstTensorScalar",
    }
    for fn in nc.m.functions:
        for bb in fn.blocks:
            incs = defaultdict(lambda: defaultdict(int))  # engine -> sem id -> n
            for inst in bb.instructions:
                eng = str(inst.engine)
                si = inst.sync_info
                if si is None:
                    continue
                waits = si.on_wait
                if waits:
                    kept = [w for w in waits
                            if not (w.wait_mode == 'sem-ge-imm'
                                    and incs[eng][w.id] >= (w.wait_value or 0))]
                    if len(kept) != len(waits):
                        si.on_wait = kept
                if type(inst).__name__ in sync_compute:
                    for up in (si.on_update or []):
                        if up.update_mode == 'sem-inc':
                            incs[eng][up.id] += up.update_value or 1

    # The kernel-tail Drain waits on every active proc's semaphore, which
    # exceeds walrus's sync-wait encoding limit. Every instruction in this
    # kernel transitively happens-before the output DMA's completion (all
    # compute feeds the out DMA; input/T1 DMAs are consumed by it), so the
    # drain only needs the out-DMA lane's wait.
    out_dma_sem = None
    for fn in nc.m.functions:
        for bb in fn.blocks:
            for inst in bb.instructions:
                if type(inst).__name__ == 'InstDMACopy':
                    outs = [str(getattr(a, 'memref', '')) for a in inst.outs]
                    if any(o.startswith('out') for o in outs):
                        for up in (inst.sync_info.on_update or []):
                            out_dma_sem = up.id
    for fn in nc.m.functions:
        for bb in fn.blocks:
            for inst in bb.instructions:
                if type(inst).__name__ == 'InstDrain' and inst.sync_info:
                    waits = inst.sync_info.on_wait or []
                    if len(waits) >= 2 and out_dma_sem is not None:
                        kept = [w for w in waits if w.id == out_dma_sem]
                        if kept:
                            inst.sync_info.on_wait = kept


def _host_prep(x, q_l_w, q_l_b, k_w, k_b, v_w, v_b, qg_w, qg_b, gamma_g, gamma_l):
    gg = float(np.asarray(gamma_g).reshape(-1)[0])
    gl = float(np.asarray(gamma_l).reshape(-1)[0])
    xf = np.ascontiguousarray(np.asarray(x, np.float32).reshape(BS, C, HW))
    q_l_w = np.asarray(q_l_w, np.float32); q_l_b = np.asarray(q_l_b, np.float32)
    k_w = np.asarray(k_w, np.float32); k_b = np.asarray(k_b, np.float32)
    v_w = np.asarray(v_w, np.float32); v_b = np.asarray(v_b, np.float32)
    qg_w = np.asarray(qg_w, np.float32); qg_b = np.asarray(qg_b, np.float32)

    # qg tiled: qgp [4096, 9] -> qgt [128, 288]; col 8 of each tile = gg*qg_b
    qgp = np.concatenate([gg * qg_w, (gg * qg_b)[:, None]], axis=1).astype(np.float32)
    qgt = np.ascontiguousarray(qgp.reshape(32, 128, 9).transpose(1, 0, 2).reshape(128, 288))

    in_maps = []
    for core in range(8):
        b, h = core // 2, core % 2
        xg = np.empty((C + 1, HW), np.float32)
        xg[:C] = xf[b]
        xg[C] = 1.0
        xh = np.empty((C + 1, HALF), np.float32)
        xh[:C] = xf[b][:, h * HALF:(h + 1) * HALF]
        xh[C] = 1.0

        cn = np.zeros((128, 576), np.float32)
        cn[:, ID0:ID0 + 128] = np.eye(128, dtype=np.float32)
        cn[:, QGT0:QGT0 + 288] = qgt
        cn[0:64, VW0:VW0 + 64] = v_w.T
        cn[64, VW0:VW0 + 64] = v_b
        cn[0:64, VW0 + 64:VW0 + 72] = k_w.T
        cn[64, VW0 + 64:VW0 + 72] = k_b
        cn[0:64, K40:K40 + 4] = k_w.T[:, 4 * h:4 * h + 4]
        cn[64, K40:K40 + 4] = k_b[4 * h:4 * h + 4]
        cn[0:64, QL0:QL0 + 8] = gl * q_l_w.T
        cn[64, QL0:QL0 + 8] = gl * q_l_b
        in_maps.append({"xg": xg, "xh": xh, "consts": cn})
    return in_maps


def kernel(**inputs):
    from concourse.bass_utils import run_bass_kernel_spmd

    if "nc" not in _CACHE:
        _CACHE["nc"] = _build_graph()
    nc = _CACHE["nc"]

    in_maps = _host_prep(**inputs)
    res = run_bass_kernel_spmd(nc, in_maps, core_ids=list(range(8)))
    outs = res.results

    full = np.empty((BS, C, HW), np.float32)
    for core in range(8):
        b, h = core // 2, core % 2
        full[b][:, h * HALF:(h + 1) * HALF] = np.asarray(outs[core]["out"])
    return full.reshape(BS, C, 64, 64)



# revision 26
# speedup vs baseline: 2.0087x; 2.0087x over previous
"""Trainium2 Bass kernel for nn_AttBlock (BS=4, C=64, H=W=64).

Math (per sample b, x viewed as [64 ch, 4096 px]):
  v  = v_w @ x + v_b                  [64, 4096]
  k  = k_w @ x + k_b                  [8, 4096]
  ql = gl*(q_l_w @ x + q_l_b)         [8, 4096]   (gamma_l folded in)
  A[c,e] = sum_q v[c,q] * gg*qg_w[q,e]            [64, 8]
  s[c]   = sum_q v[c,q] * gg*qg_b[q]              [64]
  B[c,e] = sum_q v[c,q] * k[e,q]                  [64, 8]
  out[c,p] = sum_e A[c,e]*k_g[p,e] + s[c] + sum_e B[c,e]*ql[e,p] + x[c,p]
  where k_g[p,e] = k.flat[8p+e]  (the torch .view reinterpretation)

The [4096,4096] alphas matrices are never materialized: associativity
collapses the attention into rank-8/9 factor matmuls. Additionally the
ql term re-associates: sum_e B[c,e]*ql[e,p] = sum_ch M[ch,c]*x[ch,p]
with M = (gl*q_l_w).T @ B + I (identity folded in for the +x residual,
bias via the ones row), so ql is never materialized either; the final
matmul runs directly against the own-half x.

All PE-facing tensors are bf16 (1 cycle/row vs 4 for fp32; rel-err
budget is 2e-2), accumulation in fp32 PSUM, output DMA'd from PSUM as
fp32.

Sharding: core i = (b = i//2, h = i%2): sample b, output pixel half h.
All per-core differences (sample, half via the k4 weight columns and
the xh data) live in per-core *data*, so one SPMD graph serves all 8
cores. No collectives.

k_g.T build (the mod-8 lane shuffle): kch [4,4096] --SBUF DMA-->
T1[r, 32j+m] = kch[j, 32r+m] --PE transpose--> T2[32j+m, r] (PSUM)
--strided copies--> R[e, 512j+4r+w] = T2[32j+8w+e, r]. PSUM sources
may start at any partition (SBUF APs must start 32-aligned, PSUM APs
are unconstrained), which is what makes the copies legal.
"""

import numpy as np

BS, C, HW = 4, 64, 4096
C8 = 8
HALF = HW // 2  # 2048

_CACHE = {}

# consts column layout (bf16 [128, 560])
QGT0 = 0      # [128, 288]: qg tiled; tile t at cols 9t:9t+9 (col 8 = gg*qg_b)
VW0 = 288     # [65, 72]:  [v_w.T; v_b] ++ [k_w.T; k_b]
K40 = 360     # [65, 4]:   per-core [k_w.T[:, 4h:4h+4]; k_b[4h:4h+4]]
GLW0 = 364    # [8, 65]:   G[e, ch] = gl*q_l_w[e, ch]; G[e, 64] = gl*q_l_b[e]
ID0 = 429     # [128, 128]: identity (bf16)
CN_W = 560


def _build_graph():
    import concourse.bass as bass
    import concourse.mybir as mybir
    from concourse.tile import TileContext

    BF16 = mybir.dt.bfloat16
    FP32 = mybir.dt.float32
    nc = bass.Bass("TRN2", target_bir_lowering=False)

    # ---- DRAM parameters (per-core shards) ----
    x_d = nc.declare_dram_parameter("xb", [C + 1, HW], BF16, isOutput=False)
    xh_d = nc.declare_dram_parameter("xh", [C + 1, HALF], BF16, isOutput=False)
    cn_d = nc.declare_dram_parameter("consts", [128, CN_W], BF16, isOutput=False)
    out_d = nc.declare_dram_parameter("out", [C, HALF], BF16, isOutput=True)

    with TileContext(nc) as tc:
        with (
            tc.tile_pool(name="big", bufs=1) as big,
            tc.tile_pool(name="pskq", bufs=2, space="PSUM") as pskq,
            tc.tile_pool(name="psvk", bufs=2, space="PSUM") as psvk,
            tc.tile_pool(name="psf", bufs=1, space="PSUM") as psf,
            tc.tile_pool(name="psout", bufs=2, space="PSUM") as psout,
        ):
            x_sb = big.tile([C + 1, HW], BF16, tag="x_sb")     # full sample + ones row
            xh_sb = big.tile([C + 1, HALF], BF16, tag="xh_sb")  # own-half x + ones row
            c_sb = big.tile([128, CN_W], BF16, tag="c_sb")
            kch = big.tile([4, HW], BF16, tag="kch")           # k rows 4h:4h+4, all px
            t1 = big.tile([128, 128], BF16, tag="t1")          # kch re-blocked
            R = big.tile([9, HALF], BF16, tag="R")             # rows 0:8 k_g.T, row 8 ones
            P = big.tile([9, C], BF16, tag="P")                # [A.T; s] (pg + bias row)
            pl_sb = big.tile([C8, C], BF16, tag="pl_sb")       # B.T
            M_sb = big.tile([C + 1, C], BF16, tag="M_sb")      # glW.T @ B + I (+bias row)
            # per-tile block: [v(64) | k(8) | dead(24) | qgt(9)] — the dead
            # cols pad the fused factor matmul's lhsT so pl lands at PSUM
            # rows 0:8 and pg at rows 32:41 (PSUM reads need 32-aligned
            # partition starts; the dead rows 8:32 are never read)
            F = big.tile([128, 32 * 105], BF16, tag="F")
            o_sb = big.tile([C, HALF], BF16, tag="o_sb")

            # ---- loads ----
            nc.sync.dma_start(out=c_sb[:, :], in_=cn_d[:, :])
            nc.sync.dma_start(out=x_sb[:, 0:HALF], in_=x_d[:, 0:HALF])
            nc.sync.dma_start(out=x_sb[:, HALF:HW], in_=x_d[:, HALF:HW])
            nc.sync.dma_start(out=xh_sb[:, :], in_=xh_d[:, :])

            # row 8 stays 1.0 (the 'ones' row pairing with P's bias row);
            # rows 0:8 are fully overwritten by the k_g.T copies below.
            # On DVE: everything feeding the final matmuls must sit on one
            # engine lane (each PE matmul carries a single sync wait).
            nc.vector.memset(R[:, :], 1.0)

            # Dead lhsT cols zeroed (their accumulation rows are never
            # read; zeros keep sim finite/init checks happy) + qgt tiles
            # scattered into the F slots (cols 105t+96 : 105t+105). On
            # Act: same lane as the vk pair copies, so each factor matmul
            # waits on Act alone.
            nc.gpsimd.memset(
                F.rearrange("p (t c) -> p t c", c=105)[:, :, 72:96], 0.0)
            nc.vector.tensor_copy(
                F.rearrange("p (t c) -> p t c", c=105)[:, :, 96:105],
                c_sb[:, QGT0:QGT0 + 288].rearrange("p (t c) -> p t c", c=9))

            # Tiny first matmul reading only c_sb: walrus allows a single
            # sync-wait on the LDWEIGHTS struct, so each PE instruction
            # may introduce at most ONE new semaphore lane. This observes
            # the consts DMA lane so the first real matmul waits only on x.
            dmy = psout.tile([1, 1], FP32, tag="op")
            nc.tensor.matmul(dmy[0:1, 0:1], c_sb[0:1, 0:1], c_sb[0:1, 0:1],
                             start=True, stop=True)

            ptgl = psf.tile([41, C], FP32, tag="ptgl")

            def kq_chunk(u):
                # k rows 4h:4h+4 at px chunk u (global order), ch-major.
                # Copies on Pool: the T1 re-block DMA then waits on the
                # Pool lane alone (DMAs also take only one sync wait).
                kq = pskq.tile([4, 512], FP32, tag="kq")
                nc.tensor.matmul(kq[:, :], c_sb[0:65, K40:K40 + 4],
                                 x_sb[:, 512 * u:512 * u + 512],
                                 start=True, stop=True,
                                 skip_group_check=True)
                nc.vector.tensor_copy(kch[:, 512 * u:512 * (u + 1)], kq[:, :])

            def vk_pair(p2):
                # px-major [v | k] at two 128-px tiles + fused factor matmuls
                vk = psvk.tile([128, 144], FP32, tag="vk")
                for s2 in range(2):
                    t = 2 * p2 + s2
                    nc.tensor.matmul(vk[:, 72 * s2:72 * s2 + 72],
                                     x_sb[:, 128 * t:128 * t + 128],
                                     c_sb[0:65, VW0:VW0 + 72],
                                     start=True, stop=True,
                                     skip_group_check=True)
                nc.vector.tensor_copy(
                    F.rearrange("p (t c) -> p t c", c=105)[:, 2 * p2:2 * p2 + 2, 0:72],
                    vk.rearrange("p (s c) -> p s c", c=72))
                for s2 in range(2):
                    t = 2 * p2 + s2
                    o = 105 * t
                    # rows 0:8 = B.T partial (k-cols), rows 32:41 = A.T
                    # partial (qgt cols, bias col at row 40), rows 8:32 dead
                    nc.tensor.matmul(ptgl[:, :], F[:, o + 64:o + 105],
                                     F[:, o:o + 64],
                                     start=(t == 0), stop=(t == 31))

            # PE program order is execution order (in-order engine):
            # interleave so PE never stalls on a DMA it could hide.
            for u in range(4):
                kq_chunk(u)
            # observe the Pool lane (F dead-cols memset) so the first
            # factor matmul waits on Act alone
            nc.tensor.matmul(dmy[0:1, 0:1], F[0:1, 72:73], F[0:1, 72:73],
                             start=True, stop=True, skip_group_check=True)
            for p2 in range(8):
                vk_pair(p2)
            # observe the x-chunk2 DMA lane alone so kq u=4 (which also
            # carries a pool-WAR wait) needs only one sync wait
            nc.tensor.matmul(dmy[0:1, 0:1], x_sb[0:1, HALF:HALF + 1],
                             x_sb[0:1, HALF:HALF + 1],
                             start=True, stop=True, skip_group_check=True)
            for u in range(4, 8):
                kq_chunk(u)

            # kch complete -> re-block: T1[r, 32j+m] = kch[j, 32r+m].
            # One DMA per j: a DMA pairs elements in the two APs' flat
            # iteration order, so both sides must iterate (r, m); with j
            # fixed the partition dim stays outermost in every AP.
            for j in range(4):
                nc.gpsimd.dma_start(
                    out=t1[:, 32 * j:32 * j + 32],
                    in_=kch[j:j + 1, :].rearrange("o (r m) -> o r m", m=32))

            for p2 in range(8, 16):
                vk_pair(p2)

            # observe the xh DMA lane so the final matmuls each carry a
            # single new wait
            nc.tensor.matmul(dmy[0:1, 0:1], xh_sb[0:1, 0:1], xh_sb[0:1, 0:1],
                             start=True, stop=True, skip_group_check=True)

            # ---- k_g.T build: transposes + strided copies ----
            for j in range(4):
                if j > 0:
                    # observe T1-DMA lane j alone (one new semaphore per
                    # PE instruction)
                    nc.tensor.matmul(dmy[0:1, 0:1], t1[0:1, 32 * j:32 * j + 1],
                                     t1[0:1, 32 * j:32 * j + 1],
                                     start=True, stop=True, skip_group_check=True)
                for wp in range(2):
                    t2 = psout.tile([C8, 256], BF16, tag="op")
                    for s2 in range(2):
                        w = 2 * wp + s2
                        co = 32 * j + 8 * w
                        nc.tensor.transpose(t2[:, 128 * s2:128 * s2 + 128],
                                            t1[:, co:co + 8],
                                            c_sb[:, ID0:ID0 + 128])
                    # dst cols 512j + 4r + 2wp + s2
                    dstw = R[0:8, 512 * j:512 * j + 512].rearrange(
                        "e (r s) -> e r s", s=4)[:, :, 2 * wp:2 * wp + 2]
                    srcw = t2[:, :].rearrange("e (s r) -> e r s", r=128)
                    nc.vector.tensor_copy(dstw, srcw)

            # ---- factors -> small SBUF operands ----
            nc.vector.tensor_copy(pl_sb[:, :], ptgl[0:8, :])
            nc.vector.tensor_copy(P[:, :], ptgl[32:41, :])

            # M = glW.T @ B + I  (ql term re-associated onto x; +x residual
            # via the identity; ql bias via xh's ones row x M row 64)
            M_ps = psf.tile([C + 1, C], FP32, tag="mps")
            nc.tensor.matmul(M_ps[:, :], c_sb[0:8, GLW0:GLW0 + 65], pl_sb[:, :],
                             start=True, stop=False)
            nc.tensor.matmul(M_ps[:, :], c_sb[:, ID0:ID0 + 65],
                             c_sb[:, ID0:ID0 + 64], start=False, stop=True)
            nc.vector.tensor_copy(M_sb[:, :], M_ps[:, :])

            # ---- finals: out = P.T @ R + M.T @ xh ----
            # o_sb copies alternate DVE/Pool; one out-DMA per 512-px chunk
            # so each DMA waits on a single engine lane.
            for u in range(4):
                op = psout.tile([C, 512], FP32, tag="op")
                sl = slice(u * 512, (u + 1) * 512)
                nc.tensor.matmul(op[:, :], P[:, :], R[:, sl],
                                 start=True, stop=False)
                nc.tensor.matmul(op[:, :], M_sb[:, :], xh_sb[:, sl],
                                 start=False, stop=True)
                nc.vector.tensor_copy(o_sb[:, sl], op[:, :])
                nc.sync.dma_start(out=out_d[:, sl], in_=o_sb[:, sl])

    _strip_dead_waits(nc)
    return nc


def _strip_dead_waits(nc):
    """Remove semaphore waits already implied by same-engine program order.

    Tile's dead-wait elimination pass (optimize_sems) is disabled, but
    walrus only encodes ONE sync-wait per compute instruction, so any
    instruction carrying {cross-engine wait, same-engine dead wait} fails
    codegen. A wait on sem S with value <= (number of increments to S by
    *synchronously-completing* instructions earlier on the same engine) is
    trivially satisfied when the instruction issues (engines complete
    in-order; semaphores are monotone), so it can be dropped. DMA
    increments are excluded: DMA completions are asynchronous.
    """
    from collections import defaultdict

    sync_compute = {
        "InstMatmult", "InstTensorCopy", "InstActivation", "InstMemset",
        "InstTensorTensor", "InstTensorScalarPtr", "InstTensorReduce",
        "InstCopy", "InstStreamTranspose", "InstIota", "InstTensorScalar",
    }
    for fn in nc.m.functions:
        for bb in fn.blocks:
            incs = defaultdict(lambda: defaultdict(int))  # engine -> sem id -> n
            for inst in bb.instructions:
                eng = str(inst.engine)
                si = inst.sync_info
                if si is None:
                    continue
                waits = si.on_wait
                if waits:
                    kept = [w for w in waits
                            if not (w.wait_mode == 'sem-ge-imm'
                                    and incs[eng][w.id] >= (w.wait_value or 0))]
                    if len(kept) != len(waits):
                        si.on_wait = kept
                if type(inst).__name__ in sync_compute:
                    for up in (si.on_update or []):
                        if up.update_mode == 'sem-inc':
                            incs[eng][up.id] += up.update_value or 1

    # The kernel-tail Drain waits on every active proc's semaphore, which
    # exceeds walrus's sync-wait encoding limit. Every instruction in this
    # kernel transitively happens-before the output DMAs' completion (all
    # compute feeds the out DMAs; input/T1 DMAs are consumed by them), so
    # the drain only needs the out-DMA lanes' waits.
    out_dma_sem = None
    for fn in nc.m.functions:
        for bb in fn.blocks:
            for inst in bb.instructions:
                if type(inst).__name__ == 'InstDMACopy':
                    outs = [str(getattr(a, 'memref', '')) for a in inst.outs]
                    if any(o.startswith('out') for o in outs):
                        for up in (inst.sync_info.on_update or []):
                            out_dma_sem = up.id
    for fn in nc.m.functions:
        for bb in fn.blocks:
            for inst in bb.instructions:
                if type(inst).__name__ == 'InstDrain' and inst.sync_info:
                    waits = inst.sync_info.on_wait or []
                    if len(waits) >= 2 and out_dma_sem is not None:
                        kept = [w for w in waits if w.id == out_dma_sem]
                        if kept:
                            inst.sync_info.on_wait = kept


def _host_prep(x, q_l_w, q_l_b, k_w, k_b, v_w, v_b, qg_w, qg_b, gamma_g, gamma_l):
    import ml_dtypes
    BF = ml_dtypes.bfloat16

    gg = float(np.asarray(gamma_g).reshape(-1)[0])
    gl = float(np.asarray(gamma_l).reshape(-1)[0])
    xf = np.ascontiguousarray(np.asarray(x, np.float32).reshape(BS, C, HW))
    q_l_w = np.asarray(q_l_w, np.float32); q_l_b = np.asarray(q_l_b, np.float32)
    k_w = np.asarray(k_w, np.float32); k_b = np.asarray(k_b, np.float32)
    v_w = np.asarray(v_w, np.float32); v_b = np.asarray(v_b, np.float32)
    qg_w = np.asarray(qg_w, np.float32); qg_b = np.asarray(qg_b, np.float32)

    # qg tiled: qgp [4096, 9] -> qgt [128, 288]; col 8 of each tile = gg*qg_b
    qgp = np.concatenate([gg * qg_w, (gg * qg_b)[:, None]], axis=1).astype(np.float32)
    qgt = np.ascontiguousarray(qgp.reshape(32, 128, 9).transpose(1, 0, 2).reshape(128, 288))

    in_maps = []
    for core in range(8):
        b, h = core // 2, core % 2
        xb = np.empty((C + 1, HW), np.float32)
        xb[:C] = xf[b]
        xb[C] = 1.0
        xh = np.empty((C + 1, HALF), np.float32)
        xh[:C] = xf[b][:, h * HALF:(h + 1) * HALF]
        xh[C] = 1.0

        cn = np.zeros((128, CN_W), np.float32)
        cn[:, QGT0:QGT0 + 288] = qgt
        cn[0:64, VW0:VW0 + 64] = v_w.T
        cn[64, VW0:VW0 + 64] = v_b
        cn[0:64, VW0 + 64:VW0 + 72] = k_w.T
        cn[64, VW0 + 64:VW0 + 72] = k_b
        cn[0:64, K40:K40 + 4] = k_w.T[:, 4 * h:4 * h + 4]
        cn[64, K40:K40 + 4] = k_b[4 * h:4 * h + 4]
        cn[0:8, GLW0:GLW0 + 64] = gl * q_l_w
        cn[0:8, GLW0 + 64] = gl * q_l_b
        cn[:, ID0:ID0 + 128] = np.eye(128, dtype=np.float32)
        in_maps.append({"xb": xb.astype(BF), "xh": xh.astype(BF),
                        "consts": cn.astype(BF)})
    return in_maps


def kernel(**inputs):
    from concourse.bass_utils import run_bass_kernel_spmd

    if "nc" not in _CACHE:
        _CACHE["nc"] = _build_graph()
    nc = _CACHE["nc"]

    in_maps = _host_prep(**inputs)
    res = run_bass_kernel_spmd(nc, in_maps, core_ids=list(range(8)))
    outs = res.results

    full = np.empty((BS, C, HW), np.float32)
    for core in range(8):
        b, h = core // 2, core % 2
        full[b][:, h * HALF:(h + 1) * HALF] = np.asarray(outs[core]["out"]).astype(np.float32)
    return full.reshape(BS, C, 64, 64)


# revision 29
# speedup vs baseline: 2.0475x; 1.0193x over previous
"""Trainium2 Bass kernel for nn_AttBlock (BS=4, C=64, H=W=64).

Math (per sample b, x viewed as [64 ch, 4096 px]):
  v  = v_w @ x + v_b                  [64, 4096]
  k  = k_w @ x + k_b                  [8, 4096]
  ql = gl*(q_l_w @ x + q_l_b)         [8, 4096]   (gamma_l folded in)
  A[c,e] = sum_q v[c,q] * gg*qg_w[q,e]            [64, 8]
  s[c]   = sum_q v[c,q] * gg*qg_b[q]              [64]
  B[c,e] = sum_q v[c,q] * k[e,q]                  [64, 8]
  out[c,p] = sum_e A[c,e]*k_g[p,e] + s[c] + sum_e B[c,e]*ql[e,p] + x[c,p]
  where k_g[p,e] = k.flat[8p+e]  (the torch .view reinterpretation)

The [4096,4096] alphas matrices are never materialized: associativity
collapses the attention into rank-8/9 factor matmuls. Additionally the
ql term re-associates: sum_e B[c,e]*ql[e,p] = sum_ch M[ch,c]*x[ch,p]
with M = (gl*q_l_w).T @ B + I (identity folded in for the +x residual,
bias via the ones row), so ql is never materialized either; the final
matmul runs directly against the own-half x.

All PE-facing tensors are bf16 (1 cycle/row vs 4 for fp32; rel-err
budget is 2e-2), accumulation in fp32 PSUM, output DMA'd from PSUM as
fp32.

Sharding: core i = (b = i//2, h = i%2): sample b, output pixel half h.
All per-core differences (sample, half via the k4 weight columns and
the xh data) live in per-core *data*, so one SPMD graph serves all 8
cores. No collectives.

k_g.T build (the mod-8 lane shuffle): kch [4,4096] --SBUF DMA-->
T1[r, 32j+m] = kch[j, 32r+m] --PE transpose--> T2[32j+m, r] (PSUM)
--strided copies--> R[e, 512j+4r+w] = T2[32j+8w+e, r]. PSUM sources
may start at any partition (SBUF APs must start 32-aligned, PSUM APs
are unconstrained), which is what makes the copies legal.
"""

import numpy as np

BS, C, HW = 4, 64, 4096
C8 = 8
HALF = HW // 2  # 2048

_CACHE = {}

# consts column layout (bf16 [128, 560])
QGT0 = 0      # [128, 288]: qg tiled; tile t at cols 9t:9t+9 (col 8 = gg*qg_b)
VW0 = 288     # [65, 72]:  [v_w.T; v_b] ++ [k_w.T; k_b]
K40 = 360     # [65, 4]:   per-core [k_w.T[:, 4h:4h+4]; k_b[4h:4h+4]]
GLW0 = 364    # [8, 65]:   G[e, ch] = gl*q_l_w[e, ch]; G[e, 64] = gl*q_l_b[e]
ID0 = 429     # [128, 128]: identity (bf16)
CN_W = 560


def _build_graph():
    import concourse.bass as bass
    import concourse.mybir as mybir
    from concourse.tile import TileContext

    BF16 = mybir.dt.bfloat16
    FP32 = mybir.dt.float32
    nc = bass.Bass("TRN2", target_bir_lowering=False)

    # ---- DRAM parameters (per-core shards) ----
    x_d = nc.declare_dram_parameter("xb", [C + 1, HW], BF16, isOutput=False)
    xh_d = nc.declare_dram_parameter("xh", [C + 1, HALF], BF16, isOutput=False)
    cn_d = nc.declare_dram_parameter("consts", [128, CN_W], BF16, isOutput=False)
    out_d = nc.declare_dram_parameter("out", [C, HALF], BF16, isOutput=True)

    with TileContext(nc) as tc:
        with (
            tc.tile_pool(name="big", bufs=1) as big,
            tc.tile_pool(name="pskq", bufs=2, space="PSUM") as pskq,
            tc.tile_pool(name="psvk", bufs=2, space="PSUM") as psvk,
            tc.tile_pool(name="psf", bufs=1, space="PSUM") as psf,
            tc.tile_pool(name="psout", bufs=2, space="PSUM") as psout,
        ):
            x_sb = big.tile([C + 1, HW], BF16, tag="x_sb")     # full sample + ones row
            xh_sb = big.tile([C + 1, HALF], BF16, tag="xh_sb")  # own-half x + ones row
            c_sb = big.tile([128, CN_W], BF16, tag="c_sb")
            kch = big.tile([4, HW], BF16, tag="kch")           # k rows 4h:4h+4, all px
            t1 = big.tile([128, 128], BF16, tag="t1")          # kch re-blocked
            R = big.tile([9, HALF], BF16, tag="R")             # rows 0:8 k_g.T, row 8 ones
            P = big.tile([9, C], BF16, tag="P")                # [A.T; s] (pg + bias row)
            pl_sb = big.tile([C8, C], BF16, tag="pl_sb")       # B.T
            M_sb = big.tile([C + 1, C], BF16, tag="M_sb")      # glW.T @ B + I (+bias row)
            # per-tile block: [v(64) | k(8) | dead(24) | qgt(9)] — the dead
            # cols pad the fused factor matmul's lhsT so pl lands at PSUM
            # rows 0:8 and pg at rows 32:41 (PSUM reads need 32-aligned
            # partition starts; the dead rows 8:32 are never read)
            F = big.tile([128, 32 * 105], BF16, tag="F")
            o_sb = big.tile([C, HALF], BF16, tag="o_sb")

            # ---- loads ----
            nc.sync.dma_start(out=c_sb[:, :], in_=cn_d[:, :])
            nc.sync.dma_start(out=x_sb[:, 0:HALF], in_=x_d[:, 0:HALF])
            nc.sync.dma_start(out=x_sb[:, HALF:HW], in_=x_d[:, HALF:HW])
            nc.sync.dma_start(out=xh_sb[:, :], in_=xh_d[:, :])

            # row 8 stays 1.0 (the 'ones' row pairing with P's bias row);
            # rows 0:8 are fully overwritten by the k_g.T copies below.
            # On DVE: everything feeding the final matmuls must sit on one
            # engine lane (each PE matmul carries a single sync wait).
            nc.vector.memset(R[:, :], 1.0)

            # Dead lhsT cols zeroed (their accumulation rows are never
            # read; zeros keep sim finite/init checks happy) + qgt tiles
            # scattered into the F slots (cols 105t+96 : 105t+105). On
            # Act: same lane as the vk pair copies, so each factor matmul
            # waits on Act alone.
            nc.gpsimd.memset(
                F.rearrange("p (t c) -> p t c", c=105)[:, :, 72:96], 0.0)
            nc.vector.tensor_copy(
                F.rearrange("p (t c) -> p t c", c=105)[:, :, 96:105],
                c_sb[:, QGT0:QGT0 + 288].rearrange("p (t c) -> p t c", c=9))

            # Tiny first matmul reading only c_sb: walrus allows a single
            # sync-wait on the LDWEIGHTS struct, so each PE instruction
            # may introduce at most ONE new semaphore lane. This observes
            # the consts DMA lane so the first real matmul waits only on x.
            dmy = psout.tile([1, 1], FP32, tag="op")
            nc.tensor.matmul(dmy[0:1, 0:1], c_sb[0:1, 0:1], c_sb[0:1, 0:1],
                             start=True, stop=True)

            ptgl = psf.tile([41, C], FP32, tag="ptgl")

            def kq_chunk(u):
                # k rows 4h:4h+4 at px chunk u (global order), ch-major.
                # Copies on Act: the T1 re-block DMAs then wait on the
                # Act lane alone (DMAs also take only one sync wait).
                kq = pskq.tile([4, 512], FP32, tag="kq")
                nc.tensor.matmul(kq[:, :], c_sb[0:65, K40:K40 + 4],
                                 x_sb[:, 512 * u:512 * u + 512],
                                 start=True, stop=True,
                                 skip_group_check=True)
                nc.scalar.copy(kch[:, 512 * u:512 * (u + 1)], kq[:, :])

            def vk_pair(p2):
                # px-major [v | k] at two 128-px tiles + fused factor matmuls
                vk = psvk.tile([128, 144], FP32, tag="vk")
                for s2 in range(2):
                    t = 2 * p2 + s2
                    nc.tensor.matmul(vk[:, 72 * s2:72 * s2 + 72],
                                     x_sb[:, 128 * t:128 * t + 128],
                                     c_sb[0:65, VW0:VW0 + 72],
                                     start=True, stop=True,
                                     skip_group_check=True)
                dstF = F.rearrange("p (t c) -> p t c", c=105)[:, 2 * p2:2 * p2 + 2, 0:72]
                srcF = vk.rearrange("p (s c) -> p s c", c=72)
                if p2 % 2:
                    nc.scalar.copy(dstF, srcF)
                else:
                    nc.vector.tensor_copy(dstF, srcF)
                for s2 in range(2):
                    t = 2 * p2 + s2
                    o = 105 * t
                    # rows 0:8 = B.T partial (k-cols), rows 32:41 = A.T
                    # partial (qgt cols, bias col at row 40), rows 8:32 dead
                    nc.tensor.matmul(ptgl[:, :], F[:, o + 64:o + 105],
                                     F[:, o:o + 64],
                                     start=(t == 0), stop=(t == 31))

            # PE program order is execution order (in-order engine):
            # interleave so PE never stalls on a DMA it could hide.
            for u in range(4):
                kq_chunk(u)
            # observe the Pool lane (F dead-cols memset) so the first
            # factor matmul waits on Act alone
            nc.tensor.matmul(dmy[0:1, 0:1], F[0:1, 72:73], F[0:1, 72:73],
                             start=True, stop=True, skip_group_check=True)
            for p2 in range(8):
                vk_pair(p2)
            # observe the x-chunk2 DMA lane alone so kq u=4 (which also
            # carries a pool-WAR wait) needs only one sync wait
            nc.tensor.matmul(dmy[0:1, 0:1], x_sb[0:1, HALF:HALF + 1],
                             x_sb[0:1, HALF:HALF + 1],
                             start=True, stop=True, skip_group_check=True)
            for u in range(4, 8):
                kq_chunk(u)

            # kch complete -> re-block: T1[r, 32j+m] = kch[j, 32r+m].
            # One DMA per j: a DMA pairs elements in the two APs' flat
            # iteration order, so both sides must iterate (r, m); with j
            # fixed the partition dim stays outermost in every AP.
            for j in range(4):
                nc.gpsimd.dma_start(
                    out=t1[:, 32 * j:32 * j + 32],
                    in_=kch[j:j + 1, :].rearrange("o (r m) -> o r m", m=32))

            for p2 in range(8, 16):
                vk_pair(p2)

            # observe the xh DMA lane so the final matmuls each carry a
            # single new wait
            nc.tensor.matmul(dmy[0:1, 0:1], xh_sb[0:1, 0:1], xh_sb[0:1, 0:1],
                             start=True, stop=True, skip_group_check=True)

            # ---- k_g.T build: transposes + strided copies ----
            for j in range(4):
                if j > 0:
                    # observe T1-DMA lane j alone (one new semaphore per
                    # PE instruction)
                    nc.tensor.matmul(dmy[0:1, 0:1], t1[0:1, 32 * j:32 * j + 1],
                                     t1[0:1, 32 * j:32 * j + 1],
                                     start=True, stop=True, skip_group_check=True)
                for wp in range(2):
                    t2 = psout.tile([C8, 256], BF16, tag="op")
                    for s2 in range(2):
                        w = 2 * wp + s2
                        co = 32 * j + 8 * w
                        nc.tensor.transpose(t2[:, 128 * s2:128 * s2 + 128],
                                            t1[:, co:co + 8],
                                            c_sb[:, ID0:ID0 + 128])
                    # dst cols 512j + 4r + 2wp + s2
                    dstw = R[0:8, 512 * j:512 * j + 512].rearrange(
                        "e (r s) -> e r s", s=4)[:, :, 2 * wp:2 * wp + 2]
                    srcw = t2[:, :].rearrange("e (s r) -> e r s", r=128)
                    nc.vector.tensor_copy(dstw, srcw)

            # ---- factors -> small SBUF operands ----
            nc.vector.tensor_copy(pl_sb[:, :], ptgl[0:8, :])
            nc.vector.tensor_copy(P[:, :], ptgl[32:41, :])

            # M = glW.T @ B + I  (ql term re-associated onto x; +x residual
            # via the identity; ql bias via xh's ones row x M row 64)
            M_ps = psf.tile([C + 1, C], FP32, tag="mps")
            nc.tensor.matmul(M_ps[:, :], c_sb[0:8, GLW0:GLW0 + 65], pl_sb[:, :],
                             start=True, stop=False)
            nc.tensor.matmul(M_ps[:, :], c_sb[:, ID0:ID0 + 65],
                             c_sb[:, ID0:ID0 + 64], start=False, stop=True)
            nc.vector.tensor_copy(M_sb[:, :], M_ps[:, :])

            # ---- finals: out = P.T @ R + M.T @ xh ----
            # o_sb copies alternate DVE/Pool; one out-DMA per 512-px chunk
            # so each DMA waits on a single engine lane.
            for u in range(4):
                op = psout.tile([C, 512], FP32, tag="op")
                sl = slice(u * 512, (u + 1) * 512)
                nc.tensor.matmul(op[:, :], P[:, :], R[:, sl],
                                 start=True, stop=False)
                nc.tensor.matmul(op[:, :], M_sb[:, :], xh_sb[:, sl],
                                 start=False, stop=True)
                if u % 2:
                    nc.scalar.copy(o_sb[:, sl], op[:, :])
                else:
                    nc.vector.tensor_copy(o_sb[:, sl], op[:, :])
                nc.sync.dma_start(out=out_d[:, sl], in_=o_sb[:, sl])

    _strip_dead_waits(nc)
    return nc


def _strip_dead_waits(nc):
    """Remove semaphore waits already implied by same-engine program order.

    Tile's dead-wait elimination pass (optimize_sems) is disabled, but
    walrus only encodes ONE sync-wait per compute instruction, so any
    instruction carrying {cross-engine wait, same-engine dead wait} fails
    codegen. A wait on sem S with value <= (number of increments to S by
    *synchronously-completing* instructions earlier on the same engine) is
    trivially satisfied when the instruction issues (engines complete
    in-order; semaphores are monotone), so it can be dropped. DMA
    increments are excluded: DMA completions are asynchronous.
    """
    from collections import defaultdict

    sync_compute = {
        "InstMatmult", "InstTensorCopy", "InstActivation", "InstMemset",
        "InstTensorTensor", "InstTensorScalarPtr", "InstTensorReduce",
        "InstCopy", "InstStreamTranspose", "InstIota", "InstTensorScalar",
    }
    for fn in nc.m.functions:
        for bb in fn.blocks:
            incs = defaultdict(lambda: defaultdict(int))  # engine -> sem id -> n
            for inst in bb.instructions:
                eng = str(inst.engine)
                si = inst.sync_info
                if si is None:
                    continue
                waits = si.on_wait
                if waits:
                    kept = [w for w in waits
                            if not (w.wait_mode == 'sem-ge-imm'
                                    and incs[eng][w.id] >= (w.wait_value or 0))]
                    if len(kept) != len(waits):
                        si.on_wait = kept
                if type(inst).__name__ in sync_compute:
                    for up in (si.on_update or []):
                        if up.update_mode == 'sem-inc':
                            incs[eng][up.id] += up.update_value or 1

    # The kernel-tail Drain waits on every active proc's semaphore, which
    # exceeds walrus's sync-wait encoding limit. Every instruction in this
    # kernel transitively happens-before the output DMAs' completion (all
    # compute feeds the out DMAs; input/T1 DMAs are consumed by them), so
    # the drain only needs the out-DMA lanes' waits.
    out_dma_sem = None
    for fn in nc.m.functions:
        for bb in fn.blocks:
            for inst in bb.instructions:
                if type(inst).__name__ == 'InstDMACopy':
                    outs = [str(getattr(a, 'memref', '')) for a in inst.outs]
                    if any(o.startswith('out') for o in outs):
                        for up in (inst.sync_info.on_update or []):
                            out_dma_sem = up.id
    for fn in nc.m.functions:
        for bb in fn.blocks:
            for inst in bb.instructions:
                if type(inst).__name__ == 'InstDrain' and inst.sync_info:
                    waits = inst.sync_info.on_wait or []
                    if len(waits) >= 2 and out_dma_sem is not None:
                        kept = [w for w in waits if w.id == out_dma_sem]
                        if kept:
                            inst.sync_info.on_wait = kept


def _host_prep(x, q_l_w, q_l_b, k_w, k_b, v_w, v_b, qg_w, qg_b, gamma_g, gamma_l):
    import ml_dtypes
    BF = ml_dtypes.bfloat16

    gg = float(np.asarray(gamma_g).reshape(-1)[0])
    gl = float(np.asarray(gamma_l).reshape(-1)[0])
    xf = np.ascontiguousarray(np.asarray(x, np.float32).reshape(BS, C, HW))
    q_l_w = np.asarray(q_l_w, np.float32); q_l_b = np.asarray(q_l_b, np.float32)
    k_w = np.asarray(k_w, np.float32); k_b = np.asarray(k_b, np.float32)
    v_w = np.asarray(v_w, np.float32); v_b = np.asarray(v_b, np.float32)
    qg_w = np.asarray(qg_w, np.float32); qg_b = np.asarray(qg_b, np.float32)

    # qg tiled: qgp [4096, 9] -> qgt [128, 288]; col 8 of each tile = gg*qg_b
    qgp = np.concatenate([gg * qg_w, (gg * qg_b)[:, None]], axis=1).astype(np.float32)
    qgt = np.ascontiguousarray(qgp.reshape(32, 128, 9).transpose(1, 0, 2).reshape(128, 288))

    in_maps = []
    for core in range(8):
        b, h = core // 2, core % 2
        xb = np.empty((C + 1, HW), np.float32)
        xb[:C] = xf[b]
        xb[C] = 1.0
        xh = np.empty((C + 1, HALF), np.float32)
        xh[:C] = xf[b][:, h * HALF:(h + 1) * HALF]
        xh[C] = 1.0

        cn = np.zeros((128, CN_W), np.float32)
        cn[:, QGT0:QGT0 + 288] = qgt
        cn[0:64, VW0:VW0 + 64] = v_w.T
        cn[64, VW0:VW0 + 64] = v_b
        cn[0:64, VW0 + 64:VW0 + 72] = k_w.T
        cn[64, VW0 + 64:VW0 + 72] = k_b
        cn[0:64, K40:K40 + 4] = k_w.T[:, 4 * h:4 * h + 4]
        cn[64, K40:K40 + 4] = k_b[4 * h:4 * h + 4]
        cn[0:8, GLW0:GLW0 + 64] = gl * q_l_w
        cn[0:8, GLW0 + 64] = gl * q_l_b
        cn[:, ID0:ID0 + 128] = np.eye(128, dtype=np.float32)
        in_maps.append({"xb": xb.astype(BF), "xh": xh.astype(BF),
                        "consts": cn.astype(BF)})
    return in_maps


def kernel(**inputs):
    from concourse.bass_utils import run_bass_kernel_spmd

    if "nc" not in _CACHE:
        _CACHE["nc"] = _build_graph()
    nc = _CACHE["nc"]

    in_maps = _host_prep(**inputs)
    res = run_bass_kernel_spmd(nc, in_maps, core_ids=list(range(8)))
    outs = res.results

    full = np.empty((BS, C, HW), np.float32)
    for core in range(8):
        b, h = core // 2, core % 2
        full[b][:, h * HALF:(h + 1) * HALF] = np.asarray(outs[core]["out"]).astype(np.float32)
    return full.reshape(BS, C, 64, 64)
